# revision 22
# baseline (speedup 1.0000x reference)
"""Dcls3d (learnable-position dilated conv3d) Trainium2 kernel.

Reference computes:
  K = trilinear-scatter(weight, P) -> (64, 32, 5, 5, 5)
  out = conv3d(x, K, stride 1, pad 2) + bias     x: (2,32,16,32,32) -> out: (2,64,16,32,32)

Strategy (8 cores): shard (batch n in {0,1}) x (4 chunks of 4 output d-planes).
Each core runs an implicit-GEMM direct conv, entirely in fp8 (e4m3) using
DoubleRow perf mode: every instruction contracts TWO 128-deep k-tiles
(4 w-shift replicas x 32 ic each) at 0.5 cycles/row.

Accuracy scheme (all terms share one PSUM scale S=16; measured rel
err 1.42e-2 vs the 2e-2 gate, deterministic for the fixed-seed inputs):
  - main term: fp8(S*K) x fp8(x) over all 125 taps (i=4 edge taps use
    d-shift / row-shift replica regions).
  - the 5 highest-mass (l, j) taps (~87% of sum K^2) get two same-scale
    correction terms: fp8(S*K/16) x fp8(16*(x - fp8(x))) recovers the x
    quantization error, and fp8(16*S*dK) x fp8(x/16) recovers the K
    quantization error (dK = K - fp8(S*K)/S).
  - K pre-scaled by S keeps small kernel values out of fp8 subnormals.

Implementation notes:
  - input replicas stored unpadded (32-wide rows, halo baked into each
    shift group) so every matmul rhs is one contiguous 512-run; DoubleRow
    pair APs are hand-built [128, 2, 512] with arbitrary pair stride.
  - PE p-state warm-up: zero-weight matmuls accumulate 0 into tile 0's
    PSUM while the first DMAs land, so real matmuls run at full clock.
  - epilogue (x 1/S + bias, PSUM->SBUF) runs on DVE only: stores then
    depend on a single engine's monotonic semaphore, which the tile
    framework's transitive wait reduction handles correctly.
"""

import numpy as np

import concourse.bass as bass
import concourse.bacc as bacc
import concourse.mybir as mybir
from concourse.bass_utils import run_bass_kernel_spmd
from concourse.tile import TileContext

# ---- problem constants (hardcoded per contract) ----
N, IC, D, H, W = 2, 32, 16, 32, 32
OC = 64
PAD = 2
HP = H + 2 * PAD  # 36 padded rows per plane
DSLAB = 8         # input d-planes per core (4 out + 2 halo each side)
PL = HP * W       # 1152 elements per stored plane (rows of 32)
OUTF = 4 * H * W  # 4096 outputs per (core, oc)

OFF_D = 8 * PL              # xf: main planes 0..7, then D region
OFF_J = OFF_D + 4 * PL      # then J region
XF_COLS = OFF_J + 4 * PL    # 18432
OFF_X16 = 6 * PL            # xc: dx16 planes 1..6, then x16 planes 1..6
XC_COLS = 12 * PL           # 13824

CEN = [(1, 2), (2, 1), (2, 2), (2, 3), (3, 2)]  # correction (l, j) taps
KSCALE = 16.0
NWARM = 5

# ktile kinds: ("lj",l,j) main | ("d",j) | ("j",) | ("5",) edge | ("xc",l,j)
# x-residual | ("kc",l,j) K-residual.  Pair rhs tiles: xf for lj/d/j/5,
# xc for xc/kc.
_lj = [("lj", l, j) for l in range(5) for j in range(5)]
_edge = [("d", 0), ("d", 1), ("d", 2), ("d", 3), ("d", 4), ("j",), ("5",)]
_xfk = _lj + _edge  # 32 ktiles on the xf tile
PAIRS_XF = [tuple(_xfk[i : i + 2]) for i in range(0, 32, 2)]  # 16 pairs
_xck = [("xc",) + lj for lj in CEN] + [("kc",) + lj for lj in CEN]
PAIRS_XC = [tuple(_xck[i : i + 2]) for i in range(0, 10, 2)]  # 5 pairs
PAIRS = PAIRS_XF + PAIRS_XC
NPAIR = len(PAIRS)  # 21

# pairs needing only xf main planes (phase A) vs the D/J regions (phase B)
A_XF = [pr for pr in range(16) if all(k[0] == "lj" for k in PAIRS[pr])]  # 12
B_XF = [pr for pr in range(16) if pr not in A_XF]  # 4
C_XC = list(range(16, 21))

_NC_CACHE = {}


def _construct_K(weight, P):
    """Exact numpy port of reference.construct_kernel for ks=(5,5,5)."""
    Pp = P + np.float32(2.0)
    Pf = np.floor(Pp)
    R = Pp - Pf
    P1, P2, P3 = Pf[0], Pf[1], Pf[2]
    R1, R2, R3 = R[0], R[1], R[2]
    g = np.arange(5, dtype=P.dtype)[:, None, None, None]
    aL = (g == P1) * (1.0 - R1) + (g == P1 + 1.0) * R1
    aJ = (g == P3) * (1.0 - R3) + (g == P3 + 1.0) * R3
    aI = (g == P2) * (1.0 - R2) + (g == P2 + 1.0) * R2
    K = np.einsum("ock,lock,jock,iock->oclji", weight, aL, aJ, aI, optimize=True)
    return np.ascontiguousarray(K.astype(np.float32))


def _rhs_off(kt, dd, h0):
    """Tile-local column offset of a ktile's 512-element rhs block."""
    kind = kt[0]
    if kind == "lj":
        return ((dd + kt[1]) * HP + h0 + kt[2]) * W
    if kind == "d":
        return OFF_D + (dd * HP + h0 + kt[1]) * W
    if kind == "j":
        return OFF_J + (dd * HP + h0) * W
    if kind == "5":
        return OFF_J + (dd * HP + h0 + 4) * W
    if kind == "xc":
        return ((dd + kt[1] - 1) * HP + h0 + kt[2]) * W
    if kind == "kc":
        return OFF_X16 + ((dd + kt[1] - 1) * HP + h0 + kt[2]) * W
    raise AssertionError(kt)


def _build_nc_v2(mm="hyb"):
    key = ("v3", mm)
    if key in _NC_CACHE:
        return _NC_CACHE[key]
    f32 = mybir.dt.float32
    bf16 = mybir.dt.bfloat16
    f8 = mybir.dt.float8e4
    DR = mybir.MatmulPerfMode.DoubleRow
    nc = bacc.Bacc()
    xf = nc.dram_tensor("xf", [128, XF_COLS], f8, kind="ExternalInput")
    xc = nc.dram_tensor("xc", [128, XC_COLS], f8, kind="ExternalInput")
    ktp = nc.dram_tensor("ktp", [128, NPAIR * 128], f8, kind="ExternalInput")
    bias = nc.dram_tensor("bias", [OC, 1], f32, kind="ExternalInput")
    out = nc.dram_tensor("out", [OC, OUTF], bf16, kind="ExternalOutput")

    with TileContext(nc) as tc:
        with (
            tc.tile_pool(name="const", bufs=1) as cpool,
            tc.tile_pool(name="ob", bufs=1) as opool,
            tc.tile_pool(name="psum", bufs=1, space="PSUM") as ppool,
        ):
            xf_sb = cpool.tile([128, XF_COLS], f8)
            xc_sb = cpool.tile([128, XC_COLS], f8)
            ktp_sb = cpool.tile([128, NPAIR * 128], f8)
            bias_sb = cpool.tile([OC, 1], f32)
            # chunked loads ordered by first use; subtile dep tracking lets
            # compute start per chunk
            nc.sync.dma_start(out=xf_sb[:, 0 : 2 * PL], in_=xf[:, 0 : 2 * PL])
            nc.sync.dma_start(out=ktp_sb[:, 0:768], in_=ktp[:, 0:768])
            nc.sync.dma_start(out=xf_sb[:, 2 * PL : 4 * PL], in_=xf[:, 2 * PL : 4 * PL])
            nc.sync.dma_start(out=ktp_sb[:, 768:], in_=ktp[:, 768:])
            nc.sync.dma_start(out=xf_sb[:, 4 * PL : 6 * PL], in_=xf[:, 4 * PL : 6 * PL])
            nc.sync.dma_start(out=xc_sb[:, 0 : 4 * PL], in_=xc[:, 0 : 4 * PL])
            nc.sync.dma_start(
                out=xc_sb[:, OFF_X16 : OFF_X16 + 4 * PL],
                in_=xc[:, OFF_X16 : OFF_X16 + 4 * PL],
            )
            nc.sync.dma_start(out=xf_sb[:, OFF_D:XF_COLS], in_=xf[:, OFF_D:XF_COLS])
            nc.sync.dma_start(out=xf_sb[:, 6 * PL : OFF_D], in_=xf[:, 6 * PL : OFF_D])
            nc.sync.dma_start(out=xc_sb[:, 4 * PL : 6 * PL], in_=xc[:, 4 * PL : 6 * PL])
            nc.sync.dma_start(
                out=xc_sb[:, OFF_X16 + 4 * PL :], in_=xc[:, OFF_X16 + 4 * PL :]
            )
            nc.sync.dma_start(out=bias_sb, in_=bias[:, :])

            def dr_rhs(tile_sb, offA, offB):
                ap = tile_sb[:, offA : offA + 512].unsqueeze(1)
                a = ap.ap
                a[1] = [offB - offA, 2]
                ap.ap = a
                return ap

            pss = [ppool.tile([64, 512], f32, name=f"ps{t}") for t in range(8)]
            obuf = opool.tile([64, OUTF], bf16)

            # PE p-state warm-up: zero-weight K=1 matmuls accumulating 0 into
            # tile 0's PSUM while input DMAs land.
            zs = cpool.tile([1, 576], bf16)
            nc.vector.memset(zs, 0.0)
            for wu in range(NWARM):
                nc.tensor.matmul(
                    pss[0], zs[:, 0:OC], zs[:, OC : OC + 512],
                    start=(wu == 0), stop=False, skip_group_check=True,
                )

            started = {0: True}  # tile 0's group opens with the warm-up

            def pair(t, pr, stop=False):
                dd, hh = divmod(t, 2)
                h0 = 16 * hh
                a, b = PAIRS[pr]
                tile_sb = xc_sb if a[0] in ("xc", "kc") else xf_sb
                lhsT = ktp_sb[:, pr * 128 : (pr + 1) * 128].rearrange(
                    "p (two m) -> p two m", two=2
                )
                first = t not in started
                started[t] = True
                nc.tensor.matmul(
                    pss[t], lhsT,
                    dr_rhs(tile_sb, _rhs_off(a, dd, h0), _rhs_off(b, dd, h0)),
                    start=first, stop=stop, perf_mode=DR, skip_group_check=True,
                )

            def epilogue(t, lo=0, hi=512, eng="dve"):
                dst = obuf[:, t * 512 + lo : t * 512 + hi]
                src = pss[t][:, lo:hi]
                if eng == "dve":
                    nc.vector.tensor_scalar(
                        out=dst, in0=src,
                        scalar1=1.0 / KSCALE, scalar2=bias_sb,
                        op0=mybir.AluOpType.mult, op1=mybir.AluOpType.add,
                    )
                else:
                    nc.scalar.activation(
                        out=dst, in_=src,
                        func=mybir.ActivationFunctionType.Identity,
                        bias=bias_sb, scale=1.0 / KSCALE,
                    )

            def store(lo, hi):
                nc.sync.dma_start(
                    out=out[:, lo * 512 : hi * 512], in_=obuf[:, lo * 512 : hi * 512]
                )

            # phase 1: tiles 0-3.  A-pairs (xf main planes) interleaved by
            # pair then tile to follow chunk arrival; then C-pairs (xc tile);
            # edges (D+J region, lands later) deferred per tile.
            for pr in A_XF:
                for t in (0, 1, 2, 3):
                    pair(t, pr)
            for pr in C_XC:
                for t in (0, 1, 2, 3):
                    pair(t, pr)
            for t in (0, 1, 2, 3):
                for k, pr in enumerate(B_XF):
                    pair(t, pr, stop=(k == len(B_XF) - 1))
                epilogue(t)
            store(0, 4)
            # phase 2: tiles 4-7 fully per tile
            for t in (4, 5, 6, 7):
                for pr in A_XF:
                    pair(t, pr)
                for pr in C_XC:
                    pair(t, pr)
                for k, pr in enumerate(B_XF):
                    pair(t, pr, stop=(k == len(B_XF) - 1))
                epilogue(t)
                if t == 5:
                    store(4, 6)
                if t == 6:
                    store(6, 7)
            store(7, 8)
    nc.finalize()
    _NC_CACHE[key] = nc
    return nc


def kernel(x, weight, P, bias, mm="hyb", ver="v3"):
    import ml_dtypes

    f8 = ml_dtypes.float8_e4m3
    x = np.ascontiguousarray(np.asarray(x, dtype=np.float32))
    weight = np.asarray(weight, dtype=np.float32)
    P = np.asarray(P, dtype=np.float32)
    bias = np.asarray(bias, dtype=np.float32)

    K = _construct_K(weight, P)  # (oc, ic, l, j, i)
    S = KSCALE
    K8m = (S * K).astype(f8)
    dK = K - K8m.astype(np.float32) / S
    K8xc = (S * K / 16.0).astype(f8)
    K8kc = (16.0 * S * dK).astype(f8)

    def kt_block(kt):
        blk = np.zeros((128, OC), f8)
        kind = kt[0]
        if kind in ("lj", "xc", "kc"):
            src = {"lj": K8m, "xc": K8xc, "kc": K8kc}[kind]
            for i in range(4):
                blk[i * IC : (i + 1) * IC] = src[:, :, kt[1], kt[2], i].T
        elif kind == "d":
            for lam in range(4):
                blk[lam * IC : (lam + 1) * IC] = K8m[:, :, lam, kt[1], 4].T
        elif kind == "j":
            for mu in range(4):
                blk[mu * IC : (mu + 1) * IC] = K8m[:, :, 4, mu, 4].T
        elif kind == "5":
            blk[0:IC] = K8m[:, :, 4, 4, 4].T
        return blk

    ktp_np = np.zeros((128, NPAIR * 128), f8)
    for pr, (a, b) in enumerate(PAIRS):
        ktp_np[:, pr * 128 : pr * 128 + 64] = kt_block(a)
        ktp_np[:, pr * 128 + 64 : pr * 128 + 128] = kt_block(b)

    bias_np = np.ascontiguousarray(bias.reshape(OC, 1).astype(np.float32))

    xpad = np.pad(x, ((0, 0), (0, 0), (PAD, PAD), (PAD, PAD), (PAD, PAD)))

    in_maps = []
    for ci in range(8):
        n, dc = divmod(ci, 4)
        slab = xpad[n, :, 4 * dc : 4 * dc + DSLAB]  # (32, 8, 36, 36)
        s8 = slab.astype(f8)
        sdx = (16.0 * (slab - s8.astype(np.float32))).astype(f8)
        s16 = (slab / 16.0).astype(f8)
        xf_np = np.zeros((128, XF_COLS), f8)
        xc_np = np.zeros((128, XC_COLS), f8)
        for i in range(4):
            xf_np[i * IC : (i + 1) * IC, 0 : 8 * PL] = s8[:, :, :, i : i + W].reshape(
                IC, -1
            )
            xc_np[i * IC : (i + 1) * IC, 0:OFF_X16] = sdx[
                :, 1:7, :, i : i + W
            ].reshape(IC, -1)
            xc_np[i * IC : (i + 1) * IC, OFF_X16:] = s16[:, 1:7, :, i : i + W].reshape(
                IC, -1
            )
        for lam in range(4):  # D region: plane-shift lam, w+4
            xf_np[lam * IC : (lam + 1) * IC, OFF_D : OFF_D + 4 * PL] = s8[
                :, lam : lam + 4, :, 4 : 4 + W
            ].reshape(IC, -1)
        for mu in range(4):  # J region: planes 4..7, row-shift mu, w+4
            r = np.zeros((IC, 4, HP, W), f8)
            r[:, :, : HP - mu] = s8[:, 4:8, mu:, 4 : 4 + W]
            xf_np[mu * IC : (mu + 1) * IC, OFF_J:] = r.reshape(IC, -1)
        in_maps.append({"xf": xf_np, "xc": xc_np, "ktp": ktp_np, "bias": bias_np})

    global _last_in_maps, _last_mm, _last_build
    _last_in_maps = in_maps
    _last_mm = mm
    _last_build = _build_nc_v2
    nc = _build_nc_v2(mm)
    res = run_bass_kernel_spmd(nc, in_maps, core_ids=list(range(8)))

    out = np.empty((N, OC, D, H, W), np.float32)
    for ci in range(8):
        n, dc = divmod(ci, 4)
        out[n, :, 4 * dc : 4 * dc + 4] = (
            res.results[ci]["out"].astype(np.float32).reshape(OC, 4, H, W)
        )
    return out
